# revision 22
# baseline (speedup 1.0000x reference)
"""CPSF Memcell Autoencoder on 8 Trainium2 cores — pure data parallel.

Per-core: 1 image [3,256,256]. Encoder (2 conv paths) -> memcell
(softmax retrieval over 32 slots, global delta-rule V update via
AllGather of per-core dV) -> deconv decoder back to [3,256,256].
"""
import sys
sys.path.insert(0, '/opt/trn_rl_repo')
import numpy as np
import concourse.bass as bass
import concourse.bacc as bacc
import concourse.mybir as mybir
import concourse.tile as tile
from concourse import masks
from concourse.bass_utils import run_bass_kernel_spmd

f32 = mybir.dt.float32
f32r = mybir.dt.float32r
bf16 = mybir.dt.bfloat16
AF = mybir.ActivationFunctionType
ALU = mybir.AluOpType

N_CORES = 8
N, M, S = 16, 32, 128
ALPHA = 1e-06
H = W = 256          # full res
HH = HW = 128        # half res
NSTRIP = 8           # encoder strips
SR = 32              # conv1 out rows per strip (plus 2 halo rows)
RW = 258             # padded row width in a1 buffers

_cache = {}


def _mk_ap(tile_ap, offset, dims):
    """Manual AP: partition dim from tile_ap, then free dims [[step,count],...]."""
    part = list(tile_ap.ap[0])
    return bass.AP(tile_ap.tensor, offset, [part] + [list(d) for d in dims])


def _build():
    nc = bacc.Bacc("TRN2", target_bir_lowering=False)

    # ---------------- DRAM I/O ----------------
    x_d = nc.dram_tensor("x", [3, H, W], f32r, kind="ExternalInput")
    w1s_d = nc.dram_tensor("w1s", [27, 128], f32r, kind="ExternalInput")
    w1n_d = nc.dram_tensor("w1n", [27, 16], f32r, kind="ExternalInput")
    w2s_d = nc.dram_tensor("w2s", [128, 9 * 128], f32r, kind="ExternalInput")
    w2nA_d = nc.dram_tensor("w2nA", [128, 16], f32r, kind="ExternalInput")
    w2nB_d = nc.dram_tensor("w2nB", [16, 16], f32r, kind="ExternalInput")
    ckt_d = nc.dram_tensor("ckt", [16, 32], f32r, kind="ExternalInput")
    v_d = nc.dram_tensor("vmat", [32, 128], f32r, kind="ExternalInput")
    decw_d = nc.dram_tensor("decw", [128, 9 * 12], f32r, kind="ExternalInput")
    cw3_d = nc.dram_tensor("cw3", [27, 3], f32r, kind="ExternalInput")
    b1s_d = nc.dram_tensor("b1s", [128, 1], f32, kind="ExternalInput")
    b1n_d = nc.dram_tensor("b1n", [16, 1], f32, kind="ExternalInput")
    b2s_d = nc.dram_tensor("b2s", [128, 1], f32, kind="ExternalInput")
    b2n_d = nc.dram_tensor("b2n", [16, 1], f32, kind="ExternalInput")
    bdec_d = nc.dram_tensor("bdec", [12, 1], f32, kind="ExternalInput")
    b3_d = nc.dram_tensor("b3", [3, 1], f32, kind="ExternalInput")
    zer_d = nc.dram_tensor("zer", [128, 1024], f32r, kind="ExternalInput")
    out_d = nc.dram_tensor("out", [3, H, W], f32, kind="ExternalOutput")

    with tile.TileContext(nc) as tc:
        with (
            tc.tile_pool(name="pconst", bufs=1) as pc,
            tc.tile_pool(name="ppersist", bufs=1) as pp,
            tc.tile_pool(name="pdram", bufs=1, space="DRAM") as pdram,
            tc.tile_pool(name="ps_ag", bufs=1, space="PSUM") as ps_ag_pool,
        ):
            # ------- constants -------
            w1s = pc.tile([27, 128], f32r); nc.sync.dma_start(w1s[:], w1s_d[:])
            w1n = pc.tile([27, 16], f32r); nc.sync.dma_start(w1n[:], w1n_d[:])
            w2s = pc.tile([128, 9 * 128], f32r)
            nc.sync.dma_start(w2s[:], w2s_d[:])
            w2nA = pc.tile([128, 16], f32r); nc.sync.dma_start(w2nA[:], w2nA_d[:])
            w2nB = pc.tile([16, 16], f32r); nc.sync.dma_start(w2nB[:], w2nB_d[:])
            ckt = pc.tile([16, 32], f32r); nc.sync.dma_start(ckt[:], ckt_d[:])
            vmat = pc.tile([32, 128], f32r); nc.sync.dma_start(vmat[:], v_d[:])
            decw = pc.tile([128, 9 * 12], f32r)
            nc.sync.dma_start(decw[:], decw_d[:])
            cw3 = pc.tile([27, 3], f32r); nc.sync.dma_start(cw3[:], cw3_d[:])
            b1s = pc.tile([128, 1], f32); nc.sync.dma_start(b1s[:], b1s_d[:])
            b1n = pc.tile([16, 1], f32); nc.sync.dma_start(b1n[:], b1n_d[:])
            b2s = pc.tile([128, 1], f32); nc.sync.dma_start(b2s[:], b2s_d[:])
            b2n = pc.tile([16, 1], f32); nc.sync.dma_start(b2n[:], b2n_d[:])
            bdec = pc.tile([12, 1], f32); nc.sync.dma_start(bdec[:], bdec_d[:])
            b3 = pc.tile([3, 1], f32); nc.sync.dma_start(b3[:], b3_d[:])
            ident = pc.tile([128, 128], f32)
            masks.make_identity(nc, ident[:])
            identr = pc.tile([128, 128], f32r)
            nc.vector.tensor_copy(identr[:], ident[:])

            # ------- persistent across phases -------
            w_f32 = pp.tile([128, 32 * 128], f32)       # token-major softmax weights
            vnew = pp.tile([32, 128], f32r)

            ps_ag = ps_ag_pool.tile([32, 160], f32)      # [A | G] accumulator

            # =====================  ENCODER  =====================
            with (
                tc.tile_pool(name="pE", bufs=1) as pE,
                tc.tile_pool(name="pEd", bufs=1) as pEd,
                tc.tile_pool(name="pst", bufs=2) as pst,
                tc.tile_pool(name="psA", bufs=1, space="PSUM") as psA,
                tc.tile_pool(name="psB", bufs=1, space="PSUM") as psB,
                tc.tile_pool(name="psC2", bufs=2, space="PSUM") as psC2,
            ):
                tstT = pE.tile([128, 128 * 128], bf16)   # t*^T, chunk c at cols 128c
                w_bf = pE.tile([128, 32 * 128], bf16)

                for s in range(NSTRIP):
                    y0 = 32 * s
                    # ---- im2col for conv1 (rows y0-1 .. y0+32 of conv1 output) ----
                    im1 = pEd.tile([27, 34 * 256], f32r, tag="im1")
                    for ky in range(3):
                        for kx in range(3):
                            t = ky * 3 + kx
                            r_lo = max(0, 2 - y0 - ky)
                            r_hi = min(34, 258 - y0 - ky)
                            c_lo = max(0, 1 - kx)
                            c_hi = min(256, 257 - kx)
                            nr, ncol = r_hi - r_lo, c_hi - c_lo
                            src = x_d[0:3,
                                      y0 - 2 + r_lo + ky: y0 - 2 + r_hi + ky,
                                      c_lo + kx - 1: c_hi + kx - 1]
                            nc.sync.dma_start(
                                im1[3 * t:3 * t + 3, :]
                                .rearrange("p (r c) -> p r c", r=34)[:, r_lo:r_hi, c_lo:c_hi],
                                src)
                            # zero fills
                            if r_lo > 0:
                                nc.sync.dma_start(
                                    im1[3 * t:3 * t + 3, 0:r_lo * 256],
                                    zer_d[0:3, 0:r_lo * 256] if r_lo * 256 <= 1024 else zer_d[0:3, 0:1024])
                            if r_hi < 34:
                                nrm = (34 - r_hi) * 256
                                nc.sync.dma_start(
                                    im1[3 * t:3 * t + 3, r_hi * 256: 34 * 256],
                                    zer_d[0:3, 0:nrm])
                            if c_lo > 0:
                                nc.sync.dma_start(
                                    im1[3 * t:3 * t + 3, :]
                                    .rearrange("p (r c) -> p r c", r=34)[:, :, 0:1],
                                    zer_d[0:3, 0:34].rearrange("p (r c) -> p r c", c=1))
                            if c_hi < 256:
                                nc.sync.dma_start(
                                    im1[3 * t:3 * t + 3, :]
                                    .rearrange("p (r c) -> p r c", r=34)[:, :, 255:256],
                                    zer_d[0:3, 0:34].rearrange("p (r c) -> p r c", c=1))

                    # ---- conv1 wide + narrow (17 chunks of 512 px = 2 rows) ----
                    a1s = pEd.tile([128, 34 * RW], f32r, tag="a1s")
                    # a1n flat, col-deinterleaved: row lr at [lr*260, (lr+1)*260):
                    # evens (cx=2e) at +e, odds (cx=2j+1) at +130+j
                    a1n = pE.tile([16, 34 * 260], f32r, tag="a1n")
                    for i in range(17):
                        c1 = psB.tile([128, 512], f32, tag="c1s")
                        nc.tensor.matmul(c1[:], w1s[:], im1[:, 512 * i:512 * (i + 1)],
                                         start=True, stop=True)
                        nc.scalar.activation(
                            a1s[:, :].rearrange("p (r c) -> p r c", c=RW)
                            [:, 2 * i:2 * i + 2, 1:257],
                            c1[:].rearrange("p (r c) -> p r c", r=2),
                            AF.Silu, bias=b1s[:])
                        c1n = psB.tile([16, 512], f32, tag="c1n")
                        nc.tensor.matmul(c1n[:], w1n[:], im1[:, 512 * i:512 * (i + 1)],
                                         start=True, stop=True)
                        # ACT writes de-interleaved: psum col (r,e,pe) ->
                        # a1n offset r*260 + pe*130 + e
                        nc.scalar.activation(
                            _mk_ap(a1n[:], 520 * i,
                                   [[260, 2], [1, 128], [130, 2]]),
                            c1n[:].rearrange("p (r e two) -> p r e two", r=2, two=2),
                            AF.Silu, bias=b1n[:])
                    # zero pads of a1s / a1n
                    nc.sync.dma_start(
                        a1s[:].rearrange("p (r c) -> p r c", c=RW)[:, :, 0:1],
                        zer_d[:, 0:34].rearrange("p (r c) -> p r c", c=1))
                    nc.sync.dma_start(
                        a1s[:].rearrange("p (r c) -> p r c", c=RW)[:, :, 257:258],
                        zer_d[:, 0:34].rearrange("p (r c) -> p r c", c=1))
                    if s == 0:      # conv2 zero-pad at image top: a1 row lr=0
                        nc.sync.dma_start(a1s[:, 1:257], zer_d[:, 0:256])
                        nc.sync.dma_start(a1n[:, 0:260], zer_d[0:16, 0:260])
                    if s == NSTRIP - 1:  # bottom: lr=33
                        nc.sync.dma_start(
                            a1s[:, 33 * RW + 1:33 * RW + 257], zer_d[:, 0:256])
                        nc.sync.dma_start(
                            a1n[:, 33 * 260:34 * 260], zer_d[0:16, 0:260])

                    # ---- im2col for conv2 narrow (stride 2) ----
                    im2A = pE.tile([128, 2048], f32r, tag="im2A")
                    im2B = pE.tile([16, 2048], f32r, tag="im2B")
                    for ky in range(3):
                        for kx in range(3):
                            t = ky * 3 + kx
                            dstt = im2A if t < 8 else im2B
                            prow = 16 * t if t < 8 else 0
                            dst = dstt[prow:prow + 16, :] \
                                .rearrange("p (r c) -> p r c", c=128)
                            if kx == 1:    # evens e=ox
                                src = _mk_ap(a1n[:], ky * 260,
                                             [[520, 16], [1, 128]])
                                nc.sync.dma_start(dst[:, :, :].opt(), src)
                            elif kx == 2:  # odds j=ox
                                src = _mk_ap(a1n[:], ky * 260 + 130,
                                             [[520, 16], [1, 128]])
                                nc.sync.dma_start(dst[:, :, :].opt(), src)
                            else:          # kx=0: odds j=ox-1; col ox=0 zero
                                src = _mk_ap(a1n[:], ky * 260 + 130,
                                             [[520, 16], [1, 127]])
                                nc.sync.dma_start(dst[:, :, 1:128].opt(), src)
                                nc.sync.dma_start(
                                    dst[:, :, 0:1],
                                    zer_d[0:16, 0:16].rearrange(
                                        "p (r c) -> p r c", c=1))

                    # ---- conv2 narrow -> z ----
                    z_fl = pE.tile([16, 2048], f32r, tag="z")
                    for q in range(4):
                        c2n = psB.tile([16, 512], f32, tag="c2n")
                        nc.tensor.matmul(c2n[:], w2nA[:], im2A[:, 512 * q:512 * (q + 1)],
                                         start=True, stop=False)
                        nc.tensor.matmul(c2n[:], w2nB[:], im2B[:, 512 * q:512 * (q + 1)],
                                         start=False, stop=True)
                        nc.scalar.activation(z_fl[:, 512 * q:512 * (q + 1)], c2n[:],
                                             AF.Silu, bias=b2n[:])

                    # ---- conv2 wide + silu + transpose ----
                    ps_log = psA.tile([128, 512], f32, tag="pslog")
                    for q in range(4):
                        c2 = psC2.tile([128, 512], f32, tag="c2s")
                        for t9 in range(9):
                            ky, kx = t9 // 3, t9 % 3
                            rhs = a1s[:, :].rearrange("p (r c) -> p r c", c=RW)[
                                :, 8 * q + ky: 8 * q + ky + 8: 2, kx: kx + 256: 2]
                            nc.tensor.matmul(c2[:], w2s[:, 128 * t9:128 * (t9 + 1)],
                                             rhs, start=(t9 == 0), stop=(t9 == 8))
                        ts_t = pst.tile([128, 512], f32r, tag="tst")
                        nc.scalar.activation(ts_t[:], c2[:], AF.Silu, bias=b2s[:])
                        ps_tr = psB.tile([128, 512], f32r, tag="pstr")
                        for j in range(4):
                            nc.tensor.transpose(ps_tr[:, 128 * j:128 * (j + 1)],
                                                ts_t[:, 128 * j:128 * (j + 1)], identr[:])
                        nc.vector.tensor_copy(
                            tstT[:, 2048 * s + 512 * q: 2048 * s + 512 * (q + 1)],
                            ps_tr[:].bitcast(f32))

                        # logits for the 4 chunks of this q
                        for j in range(4):
                            c = 16 * s + 4 * q + j
                            nc.tensor.matmul(
                                ps_log[:, 32 * (4 * q + j):32 * (4 * q + j) + 32],
                                z_fl[0:16, 512 * q + 128 * j: 512 * q + 128 * (j + 1)],
                                ckt[:], start=True, stop=True)

                    # ---- softmax over 32 slots (free dim), 16 chunks at once ----
                    e_st = pst.tile([128, 512], f32, tag="est")
                    nc.scalar.activation(e_st[:], ps_log[:], AF.Exp)
                    den = pst.tile([128, 16], f32, tag="den")
                    nc.vector.tensor_reduce(
                        den[:], e_st[:].rearrange("p (c k) -> p c k", k=32),
                        mybir.AxisListType.X, ALU.add)
                    rec = pst.tile([128, 16], f32, tag="rec")
                    nc.vector.reciprocal(rec[:], den[:])
                    wslice = w_f32[:, 512 * s:512 * (s + 1)]
                    nc.vector.tensor_tensor(
                        wslice.rearrange("p (c k) -> p c k", k=32),
                        e_st[:].rearrange("p (c k) -> p c k", k=32),
                        rec[:].rearrange("p (c k) -> p c k", k=1).broadcast_to([128, 16, 32]),
                        ALU.mult)
                    nc.vector.tensor_copy(w_bf[:, 512 * s:512 * (s + 1)], wslice)

                    # ---- A|G accumulation over this strip's 16 chunks ----
                    for j in range(16):
                        c = 16 * s + j
                        lhs = w_bf[:, 32 * c:32 * c + 32]
                        nc.tensor.matmul(ps_ag[:, 0:128], lhs,
                                         tstT[:, 128 * c:128 * (c + 1)],
                                         start=(c == 0), stop=(c == 127))
                        nc.tensor.matmul(ps_ag[:, 128:160], lhs, lhs,
                                         start=(c == 0), stop=(c == 127))

                # ---- dV + collective (still inside encoder pools) ----
                a_sb = pst.tile([32, 128], f32, tag="asb", bufs=1)
                nc.vector.tensor_copy(a_sb[:], ps_ag[:, 0:128])
                g_sb = pst.tile([32, 32], f32r, tag="gsb", bufs=1)
                nc.vector.tensor_copy(g_sb[:], ps_ag[:, 128:160])
                ps_gv_t = psC2.tile([128, 512], f32, tag="c2s", name="psgv")
                ps_gv = ps_gv_t[0:32, 0:128]
                nc.tensor.matmul(ps_gv[:], g_sb[:], vmat[:], start=True, stop=True)
                dv_sb = pst.tile([32, 128], f32, tag="dvsb", bufs=1)
                nc.vector.tensor_sub(dv_sb[:], a_sb[:], ps_gv[:])
                dv_in = pdram.tile([32, 128], f32)
                dv_out = pdram.tile([32 * N_CORES, 128], f32)
                nc.sync.dma_start(dv_in[:], dv_sb[:])
                nc.gpsimd.collective_compute(
                    "AllGather", ALU.bypass,
                    replica_groups=[list(range(N_CORES))],
                    ins=[dv_in.opt()], outs=[dv_out.opt()])
                gath = pst.tile([32, 8 * 128], f32, tag="gath", bufs=1)
                nc.sync.dma_start(
                    gath[:].rearrange("p (r c) -> p r c", r=N_CORES),
                    dv_out[:].rearrange("(r p) c -> p r c", p=32))
                nc.vector.tensor_add(gath[:, 0:512], gath[:, 0:512], gath[:, 512:1024])
                nc.vector.tensor_add(gath[:, 0:256], gath[:, 0:256], gath[:, 256:512])
                nc.vector.tensor_add(gath[:, 0:128], gath[:, 0:128], gath[:, 128:256])
                nc.vector.scalar_tensor_tensor(
                    vnew[:], gath[:, 0:128], ALPHA, vmat[:],
                    op0=ALU.mult, op1=ALU.add)

            # =====================  DECODER  =====================
            with (
                tc.tile_pool(name="pD", bufs=1) as pD,
                tc.tile_pool(name="pDd", bufs=2) as pDd,
                tc.tile_pool(name="pst2", bufs=3) as pst2,
                tc.tile_pool(name="psC", bufs=2, space="PSUM") as psC,
                tc.tile_pool(name="psD", bufs=1, space="PSUM") as psD,
            ):
                # ---- w slot-major via PE transpose ----
                w_sT = pD.tile([32, 16384], f32r)
                for g in range(32):           # 4 chunks per psum bank
                    ps_wt = psD.tile([32, 512], f32, tag="pswt")
                    for j in range(4):
                        c = 4 * g + j
                        nc.tensor.transpose(ps_wt[:, 128 * j:128 * (j + 1)],
                                            w_f32[:, 32 * c:32 * c + 32], ident[:])
                    nc.vector.tensor_copy(w_sT[:, 512 * g:512 * (g + 1)], ps_wt[:])

                # ---- t_read^T -> d0m (padded [130,130]) ----
                d0m = pD.tile([128, 130 * 130], f32r)
                nc.sync.dma_start(d0m[:, 0:130], zer_d[:, 0:130])
                nc.sync.dma_start(d0m[:, 129 * 130:130 * 130], zer_d[:, 0:130])
                nc.sync.dma_start(
                    d0m[:].rearrange("p (r c) -> p r c", c=130)[:, :, 0:1],
                    zer_d[:, 0:130].rearrange("p (r c) -> p r c", c=1))
                nc.sync.dma_start(
                    d0m[:].rearrange("p (r c) -> p r c", c=130)[:, :, 129:130],
                    zer_d[:, 0:130].rearrange("p (r c) -> p r c", c=1))
                for q in range(32):
                    ps_rd = psC.tile([128, 512], f32, tag="psrd")
                    nc.tensor.matmul(ps_rd[:], vnew[:], w_sT[:, 512 * q:512 * (q + 1)],
                                     start=True, stop=True)
                    nc.vector.tensor_copy(
                        d0m[:].rearrange("p (r c) -> p r c", c=130)
                        [:, 4 * q + 1:4 * q + 5, 1:129],
                        ps_rd[:].rearrange("p (r c) -> p r c", r=4))

                # ---- deconv: 9 shifts -> out12 [12, 16384], packed dec1p ----
                dec1p = pD.tile([96, 16 * 128], f32r)
                for q in range(32):
                    ps_dec = psC.tile([12, 512], f32, tag="psdec")
                    for t9 in range(9):
                        dy, dx = t9 // 3 - 1, t9 % 3 - 1
                        rhs = d0m[:].rearrange("p (r c) -> p r c", c=130)[
                            :, 4 * q + 1 + dy:4 * q + 5 + dy, 1 + dx:129 + dx]
                        nc.tensor.matmul(ps_dec[:], decw[:, 12 * t9:12 * (t9 + 1)],
                                         rhs, start=(t9 == 0), stop=(t9 == 8))
                    stg = pst2.tile([12, 512], f32r, tag="stdec")
                    nc.scalar.activation(stg[:], ps_dec[:], AF.Silu, bias=bdec[:])
                    # pack: partition = row12 + 12*(I//16); free = (I%16)*128+J
                    nc.sync.dma_start(
                        dec1p[12 * (q // 4):12 * (q // 4) + 12,
                              (4 * q % 16) * 128:(4 * q % 16) * 128 + 512],
                        stg[:])

                # ---- conv3 over full-res image, strips of 16 rows ----
                # dec1p[(a*2+b)*3+o + 12*(I//16), (I%16)*128 + J] = img[o, 2I+a, 2J+b]
                SR3 = 16                 # out rows per conv3 strip
                U3 = SR3 // 2            # half-res rows per strip
                for sp in range(256 // SR3):
                    im3 = pDd.tile([27, SR3 * 256], f32r, tag="im3")
                    for ky in range(3):
                        for kx in range(3):
                            t = ky * 3 + kx
                            for vp in range(2):      # out-row parity class
                                for xp in range(2):  # out-col parity class
                                    # derive source (a,I), (b,J) runs
                                    if vp == 0:
                                        a = (ky - 1) % 2
                                        ioff = -1 if ky == 0 else 0
                                    else:
                                        a = ky % 2
                                        ioff = 1 if ky == 2 else 0
                                    if xp == 0:
                                        b = (kx - 1) % 2
                                        joff = -1 if kx == 0 else 0
                                    else:
                                        b = kx % 2
                                        joff = 1 if kx == 2 else 0
                                    prow = (a * 2 + b) * 3
                                    # u in [0,U3): I = U3*sp + u + ioff
                                    # n in [0,128): J = n + joff
                                    u_lo = max(0, -(U3 * sp + ioff))
                                    u_hi = min(U3, 128 - U3 * sp - ioff)
                                    n_lo = max(0, -joff)
                                    n_hi = min(128, 128 - joff)
                                    # im3 col order: free = v*256 + xp*128 + n
                                    # (out pixel X = 2n + xp)
                                    dst3 = im3[3 * t:3 * t + 3, :].rearrange(
                                        "p (r c) -> p r c", c=256)
                                    xb = 128 * xp
                                    # zero-fill clipped edges
                                    if u_lo > 0:   # row v=vp (u=0) zero
                                        nc.sync.dma_start(
                                            dst3[:, vp:vp + 1, xb:xb + 128],
                                            zer_d[0:3, 0:128].rearrange(
                                                "p (r c) -> p r c", r=1))
                                    if u_hi < U3:  # last row of class zero
                                        nc.sync.dma_start(
                                            dst3[:, SR3 - 2 + vp:SR3 - 1 + vp,
                                                 xb:xb + 128],
                                            zer_d[0:3, 0:128].rearrange(
                                                "p (r c) -> p r c", r=1))
                                    if n_lo > 0:
                                        nc.sync.dma_start(
                                            dst3[:, vp::2, xb:xb + 1],
                                            zer_d[0:3, 0:U3].rearrange(
                                                "p (r c) -> p r c", c=1))
                                    if n_hi < 128:
                                        nc.sync.dma_start(
                                            dst3[:, vp::2, xb + 127:xb + 128],
                                            zer_d[0:3, 0:U3].rearrange(
                                                "p (r c) -> p r c", c=1))
                                    # main, split so (I//16) is constant per run
                                    for u0, u1 in _group_runs(u_lo, u_hi, U3 * sp, ioff):
                                        grp = (U3 * sp + u0 + ioff) // 16
                                        i_in = (U3 * sp + u0 + ioff) % 16
                                        src = dec1p[prow + 12 * grp:prow + 12 * grp + 3, :] \
                                            .rearrange("p (r c) -> p r c", c=128)[
                                                :, i_in:i_in + (u1 - u0), n_lo + joff:n_hi + joff]
                                        nc.sync.dma_start(
                                            dst3[:, 2 * u0 + vp:2 * (u1 - 1) + vp + 1:2,
                                                 xb + n_lo:xb + n_hi],
                                            src)
                    # conv3 matmuls + silu -> out
                    for i in range(SR3 * 256 // 512):
                        c3 = psC.tile([3, 512], f32, tag="c3")
                        nc.tensor.matmul(c3[:], cw3[:], im3[:, 512 * i:512 * (i + 1)],
                                         start=True, stop=True)
                        og = pst2.tile([3, 512], f32, tag="og")
                        nc.scalar.activation(og[:], c3[:], AF.Silu, bias=b3[:])
                        for vv in range(2):
                            for xpp in range(2):
                                nc.sync.dma_start(
                                    out_d[0:3, SR3 * sp + 2 * i + vv,
                                          xpp:254 + xpp + 1:2],
                                    og[:, 256 * vv + 128 * xpp:
                                          256 * vv + 128 * xpp + 128])

    nc.compile()
    return nc


def _group_runs(u_lo, u_hi, ibase, ioff):
    """Split u-range so (ibase + u + ioff)//16 is constant per run."""
    runs = []
    u = u_lo
    while u < u_hi:
        grp = (ibase + u + ioff) // 16
        ue = u
        while ue < u_hi and (ibase + ue + ioff) // 16 == grp:
            ue += 1
        runs.append((u, ue))
        u = ue
    return runs


def _prep_weights(i):
    """Host-side weight layout prep. i = dict of full inputs."""
    f = np.float32
    w1s = np.ascontiguousarray(
        i['e0s_w1'].transpose(2, 3, 1, 0).reshape(27, 128)).astype(f)
    w1n = np.ascontiguousarray(
        i['e0n_w1'].transpose(2, 3, 1, 0).reshape(27, 16)).astype(f)
    w2s = np.ascontiguousarray(
        i['e0s_w2'].transpose(1, 2, 3, 0).reshape(128, 9 * 128)).astype(f)
    w2n = np.ascontiguousarray(
        i['e0n_w2'].transpose(2, 3, 1, 0).reshape(9, 16, 16)).astype(f)
    w2nA = w2n[0:8].reshape(128, 16).copy()
    w2nB = w2n[8].copy()
    ckt = (i['cell_k'].T * np.float32(0.25)).astype(f).copy()   # [16,32], /sqrt(16)
    vmat = i['cell_v'].astype(f).copy()
    # deconv: shift s=(dy,dx); decw[s][c, (a*2+b)*3+o] = W[c,o,ky(a,u),kx(b,v)]
    dw = i['d0_dw']  # [128, 3, 4, 4]
    decw = np.zeros((9, 128, 12), f)  # reshaped to [128, 108] below
    for a in range(2):
        for u in range(2):
            ky = (1, 3)[u] if a == 0 else (0, 2)[u]
            dy = (0, -1)[u] if a == 0 else (1, 0)[u]
            for b in range(2):
                for v in range(2):
                    kx = (1, 3)[v] if b == 0 else (0, 2)[v]
                    dx = (0, -1)[v] if b == 0 else (1, 0)[v]
                    sidx = (dy + 1) * 3 + (dx + 1)
                    for o in range(3):
                        decw[sidx, :, (a * 2 + b) * 3 + o] += dw[:, o, ky, kx]
    cw3 = np.ascontiguousarray(
        i['d0_cw'].transpose(2, 3, 1, 0).reshape(27, 3)).astype(f)
    bdec = np.zeros((12, 1), f)
    for ab in range(4):
        bdec[3 * ab:3 * ab + 3, 0] = i['d0_db']
    return dict(
        w1s=w1s, w1n=w1n, w2s=w2s, w2nA=w2nA, w2nB=w2nB, ckt=ckt, vmat=vmat,
        decw=np.ascontiguousarray(decw.transpose(1, 0, 2).reshape(128, 108)),
        cw3=cw3,
        b1s=i['e0s_b1'].reshape(128, 1).astype(f),
        b1n=i['e0n_b1'].reshape(16, 1).astype(f),
        b2s=i['e0s_b2'].reshape(128, 1).astype(f),
        b2n=i['e0n_b2'].reshape(16, 1).astype(f),
        bdec=bdec, b3=i['d0_cb'].reshape(3, 1).astype(f),
        zer=np.zeros((128, 1024), f),
    )


_last = {}


def last_exec_ns():
    return _last.get('ns')


def _get_runner():
    """Cached jitted SPMD callable over 8 cores (traced once)."""
    if 'runner' in _cache:
        return _cache['runner']
    import jax
    from jax.sharding import Mesh, PartitionSpec
    from jax.experimental.shard_map import shard_map
    from concourse import bass2jax, mybir as _mb
    nc = _cache['nc']
    bass2jax.install_neuronx_cc_hook()
    partition_name = nc.partition_id_tensor.name if nc.partition_id_tensor else None
    in_names, out_names, out_avals, zero_outs = [], [], [], []
    for alloc in nc.m.functions[0].allocations:
        if not isinstance(alloc, _mb.MemoryLocationSet):
            continue
        name = alloc.memorylocations[0].name
        if alloc.kind == "ExternalInput":
            if name != partition_name:
                in_names.append(name)
        elif alloc.kind == "ExternalOutput":
            shape = tuple(alloc.tensor_shape)
            dtype = _mb.dt.np(alloc.dtype)
            out_names.append(name)
            out_avals.append(jax.core.ShapedArray(shape, dtype))
            zero_outs.append(np.zeros(shape, dtype))
    n_params = len(in_names)
    n_outs = len(out_avals)
    all_names = list(in_names) + list(out_names)
    if partition_name is not None:
        all_names.append(partition_name)
    donate = tuple(range(n_params, n_params + n_outs))

    def _body(*args):
        operands = list(args)
        if partition_name is not None:
            operands.append(bass2jax.partition_id_tensor())
        outs = bass2jax._bass_exec_p.bind(
            *operands, out_avals=tuple(out_avals), in_names=tuple(all_names),
            out_names=tuple(out_names), lowering_input_output_aliases=(),
            sim_require_finite=True, sim_require_nnan=True, nc=nc)
        return tuple(outs)

    devices = jax.devices()[:N_CORES]
    mesh = Mesh(np.asarray(devices), ("core",))
    sharded = jax.jit(
        shard_map(_body, mesh=mesh,
                  in_specs=(PartitionSpec("core"),) * (n_params + n_outs),
                  out_specs=(PartitionSpec("core"),) * n_outs,
                  check_rep=False),
        keep_unused=True)

    from jax.sharding import NamedSharding
    sh = NamedSharding(mesh, PartitionSpec("core"))
    _cache['sharding'] = sh
    _cache['devices'] = devices
    _cache['runner'] = (sharded, in_names, out_names, out_avals, zero_outs)
    return _cache['runner']


def _make_global(per_core_arrs):
    """Assemble a sharded global array from per-core numpy shards (no
    on-device slicing)."""
    import jax
    sh = _cache['sharding']
    devices = _cache['devices']
    a0 = np.asarray(per_core_arrs[0])
    global_shape = (len(per_core_arrs) * a0.shape[0], *a0.shape[1:])
    bufs = [jax.device_put(np.ascontiguousarray(a), d)
            for a, d in zip(per_core_arrs, devices)]
    return jax.make_array_from_single_device_arrays(global_shape, sh, bufs)


def _run_fast(in_maps):
    import jax
    sharded, in_names, out_names, out_avals, zero_outs = _get_runner()
    if 'dev_zeros' not in _cache:
        _cache['dev_zeros'] = [
            _make_global([np.zeros(z.shape, z.dtype)] * N_CORES)
            for z in zero_outs]
    n_cores = len(in_maps)
    gin = [_make_global([in_maps[c][nm] for c in range(n_cores)])
           for nm in in_names]
    outs = sharded(*gin, *_cache['dev_zeros'])
    return [{nm: np.asarray(outs[i]).reshape(n_cores, *out_avals[i].shape)[c]
             for i, nm in enumerate(out_names)} for c in range(n_cores)]


def _get_chain_runner(n_chain):
    """Jitted callable running the kernel n_chain times serially on-device
    (each iteration's 'out' feeds the next one's out-buffer operand)."""
    key = f'chain{n_chain}'
    if key in _cache:
        return _cache[key]
    import jax
    from jax.sharding import Mesh, PartitionSpec
    from jax.experimental.shard_map import shard_map
    from concourse import bass2jax, mybir as _mb
    nc = _cache['nc']
    bass2jax.install_neuronx_cc_hook()
    partition_name = nc.partition_id_tensor.name if nc.partition_id_tensor else None
    in_names, out_names, out_avals = [], [], []
    for alloc in nc.m.functions[0].allocations:
        if not isinstance(alloc, _mb.MemoryLocationSet):
            continue
        name = alloc.memorylocations[0].name
        if alloc.kind == "ExternalInput":
            if name != partition_name:
                in_names.append(name)
        elif alloc.kind == "ExternalOutput":
            out_names.append(name)
            out_avals.append(jax.core.ShapedArray(
                tuple(alloc.tensor_shape), _mb.dt.np(alloc.dtype)))
    n_params = len(in_names)
    all_names = list(in_names) + list(out_names)
    if partition_name is not None:
        all_names.append(partition_name)
    oi = out_names.index("out")

    def _body(*args):
        ins = list(args[:n_params])
        outbufs = list(args[n_params:])
        for _ in range(n_chain):
            operands = ins + outbufs
            if partition_name is not None:
                operands.append(bass2jax.partition_id_tensor())
            res = bass2jax._bass_exec_p.bind(
                *operands, out_avals=tuple(out_avals), in_names=tuple(all_names),
                out_names=tuple(out_names), lowering_input_output_aliases=(),
                sim_require_finite=True, sim_require_nnan=True, nc=nc)
            outbufs[oi] = res[oi]       # serialize iterations
        return tuple(res)

    devices = jax.devices()[:N_CORES]
    mesh = Mesh(np.asarray(devices), ("core",))
    n_outs = len(out_avals)
    fn = jax.jit(
        shard_map(_body, mesh=mesh,
                  in_specs=(PartitionSpec("core"),) * (n_params + n_outs),
                  out_specs=(PartitionSpec("core"),) * n_outs,
                  check_rep=False),
        keep_unused=True)
    _cache[key] = fn
    return fn


def _build_tiny():
    nc = bacc.Bacc("TRN2", target_bir_lowering=False, name="tiny")
    xi = nc.dram_tensor("xi", [128, 128], f32, kind="ExternalInput")
    xo = nc.dram_tensor("xo", [128, 128], f32, kind="ExternalOutput")
    with tile.TileContext(nc) as tc:
        with tc.tile_pool(name="sb", bufs=1) as sb:
            t = sb.tile([128, 128], f32)
            nc.sync.dma_start(t[:], xi[:])
            nc.sync.dma_start(xo[:], t[:])
    nc.compile()
    return nc


def bench_hw(n_iter=12, **inputs):
    """Estimate device exec time: full-kernel min wall minus trivial-kernel
    min wall (same 8-core dispatch path)."""
    import time as _t, jax
    from jax.sharding import Mesh, PartitionSpec
    from jax.experimental.shard_map import shard_map
    from concourse import bass2jax
    if 'nc' not in _cache:
        _cache['nc'] = _build()
    shared = _prep_weights({k: np.asarray(v) for k, v in inputs.items()})
    x = np.asarray(inputs['x'], dtype=np.float32)
    in_maps = [dict(shared, x=np.ascontiguousarray(x[c])) for c in range(N_CORES)]
    sharded, in_names, out_names, out_avals, zero_outs = _get_runner()
    gin = [_make_global([in_maps[c][nm] for c in range(N_CORES)])
           for nm in in_names]
    gz = [_make_global([np.zeros(z.shape, z.dtype)] * N_CORES)
          for z in zero_outs]

    def mintime(fn, args):
        ts = []
        for _ in range(n_iter):
            t0 = _t.perf_counter()
            o = fn(*args)
            jax.block_until_ready(o)
            ts.append(_t.perf_counter() - t0)
        return min(ts), ts

    tfull, ts_full = mintime(sharded, (*gin, *gz))

    if 'tiny_fn' not in _cache:
        ncT = _build_tiny()
        bass2jax.install_neuronx_cc_hook()
        pn = ncT.partition_id_tensor.name if ncT.partition_id_tensor else None

        def _tb(xi, xoz):
            ops = [xi, xoz]
            if pn is not None:
                ops.append(bass2jax.partition_id_tensor())
            names = ["xi", "xo"] + ([pn] if pn else [])
            return tuple(bass2jax._bass_exec_p.bind(
                *ops,
                out_avals=(jax.core.ShapedArray((128, 128), np.float32),),
                in_names=tuple(names), out_names=("xo",),
                lowering_input_output_aliases=(),
                sim_require_finite=True, sim_require_nnan=True, nc=ncT))
        mesh = Mesh(np.asarray(_cache['devices']), ("core",))
        _cache['tiny_fn'] = jax.jit(shard_map(
            _tb, mesh=mesh, in_specs=(PartitionSpec("core"),) * 2,
            out_specs=(PartitionSpec("core"),), check_rep=False),
            keep_unused=True)
        _cache['tiny_in'] = (
            _make_global([np.zeros((128, 128), np.float32)] * N_CORES),
            _make_global([np.zeros((128, 128), np.float32)] * N_CORES))
    ttiny, ts_tiny = mintime(_cache['tiny_fn'], _cache['tiny_in'])
    return max(0.0, tfull - ttiny), tfull, ttiny


def bench(n_iter=20, **inputs):
    """Min wall time of the on-device executable (inputs pre-staged)."""
    import time as _t, jax
    if 'nc' not in _cache:
        _cache['nc'] = _build()
    shared = _prep_weights({k: np.asarray(v) for k, v in inputs.items()})
    x = np.asarray(inputs['x'], dtype=np.float32)
    in_maps = [dict(shared, x=np.ascontiguousarray(x[c])) for c in range(N_CORES)]
    sharded, in_names, out_names, out_avals, zero_outs = _get_runner()
    gin = [_make_global([in_maps[c][nm] for c in range(N_CORES)])
           for nm in in_names]
    times = []
    for it in range(n_iter):
        t0 = _t.perf_counter()
        outs = sharded(*gin, *_cache['dev_zeros'])
        jax.block_until_ready(outs)
        times.append(_t.perf_counter() - t0)
    return min(times), times


def kernel(**inputs):
    if 'nc' not in _cache:
        _cache['nc'] = _build()
    nc = _cache['nc']
    shared = _prep_weights({k: np.asarray(v) for k, v in inputs.items()})
    x = np.asarray(inputs['x'], dtype=np.float32)
    in_maps = [dict(shared, x=np.ascontiguousarray(x[c])) for c in range(N_CORES)]
    res = _run_fast(in_maps)
    out = np.stack([res[c]["out"] for c in range(N_CORES)], axis=0)
    return out


# revision 24
# speedup vs baseline: 28.7114x; 28.7114x over previous
"""CPSF Memcell Autoencoder on 8 Trainium2 cores — pure data parallel.

Per-core: 1 image [3,256,256]. Encoder (2 conv paths) -> memcell
(softmax retrieval over 32 slots, global delta-rule V update via
AllGather of per-core dV) -> deconv decoder back to [3,256,256].
"""
import sys
sys.path.insert(0, '/opt/trn_rl_repo')
import numpy as np
import concourse.bass as bass
import concourse.bacc as bacc
import concourse.mybir as mybir
import concourse.tile as tile
from concourse import masks
from concourse.bass_utils import run_bass_kernel_spmd

f32 = mybir.dt.float32
f32r = mybir.dt.float32r
bf16 = mybir.dt.bfloat16
AF = mybir.ActivationFunctionType
ALU = mybir.AluOpType

N_CORES = 8
N, M, S = 16, 32, 128
ALPHA = 1e-06
H = W = 256          # full res
HH = HW = 128        # half res
NSTRIP = 8           # encoder strips
SR = 32              # conv1 out rows per strip (plus 2 halo rows)
RW = 258             # padded row width in a1 buffers

_cache = {}


def _mk_ap(tile_ap, offset, dims):
    """Manual AP: partition dim from tile_ap, then free dims [[step,count],...]."""
    part = list(tile_ap.ap[0])
    return bass.AP(tile_ap.tensor, offset, [part] + [list(d) for d in dims])


def _build():
    nc = bacc.Bacc("TRN2", target_bir_lowering=False)

    # ---------------- DRAM I/O ----------------
    x_d = nc.dram_tensor("x", [3, H, W], f32r, kind="ExternalInput")
    w1s_d = nc.dram_tensor("w1s", [27, 128], f32r, kind="ExternalInput")
    w1n_d = nc.dram_tensor("w1n", [27, 16], f32r, kind="ExternalInput")
    w2s_d = nc.dram_tensor("w2s", [128, 9 * 128], f32r, kind="ExternalInput")
    w2nA_d = nc.dram_tensor("w2nA", [128, 16], f32r, kind="ExternalInput")
    w2nB_d = nc.dram_tensor("w2nB", [16, 16], f32r, kind="ExternalInput")
    ckt_d = nc.dram_tensor("ckt", [16, 32], f32r, kind="ExternalInput")
    v_d = nc.dram_tensor("vmat", [32, 128], f32r, kind="ExternalInput")
    decw_d = nc.dram_tensor("decw", [128, 9 * 12], f32r, kind="ExternalInput")
    cw3_d = nc.dram_tensor("cw3", [27, 3], f32r, kind="ExternalInput")
    b1s_d = nc.dram_tensor("b1s", [128, 1], f32, kind="ExternalInput")
    b1n_d = nc.dram_tensor("b1n", [16, 1], f32, kind="ExternalInput")
    b2s_d = nc.dram_tensor("b2s", [128, 1], f32, kind="ExternalInput")
    b2n_d = nc.dram_tensor("b2n", [16, 1], f32, kind="ExternalInput")
    bdec_d = nc.dram_tensor("bdec", [12, 1], f32, kind="ExternalInput")
    b3_d = nc.dram_tensor("b3", [3, 1], f32, kind="ExternalInput")
    zer_d = nc.dram_tensor("zer", [128, 1024], f32r, kind="ExternalInput")
    out_d = nc.dram_tensor("out", [3, H, W], f32, kind="ExternalOutput")

    with tile.TileContext(nc) as tc:
        with (
            tc.tile_pool(name="pconst", bufs=1) as pc,
            tc.tile_pool(name="ppersist", bufs=1) as pp,
            tc.tile_pool(name="pdram", bufs=1, space="DRAM") as pdram,
            tc.tile_pool(name="ps_ag", bufs=1, space="PSUM") as ps_ag_pool,
        ):
            # ------- constants -------
            w1s = pc.tile([27, 128], f32r); nc.sync.dma_start(w1s[:], w1s_d[:])
            w1n = pc.tile([27, 16], f32r); nc.sync.dma_start(w1n[:], w1n_d[:])
            w2s = pc.tile([128, 9 * 128], f32r)
            nc.sync.dma_start(w2s[:], w2s_d[:])
            w2nA = pc.tile([128, 16], f32r); nc.sync.dma_start(w2nA[:], w2nA_d[:])
            w2nB = pc.tile([16, 16], f32r); nc.sync.dma_start(w2nB[:], w2nB_d[:])
            ckt = pc.tile([16, 32], f32r); nc.sync.dma_start(ckt[:], ckt_d[:])
            vmat = pc.tile([32, 128], f32r); nc.sync.dma_start(vmat[:], v_d[:])
            decw = pc.tile([128, 9 * 12], f32r)
            nc.sync.dma_start(decw[:], decw_d[:])
            cw3 = pc.tile([27, 3], f32r); nc.sync.dma_start(cw3[:], cw3_d[:])
            b1s = pc.tile([128, 1], f32); nc.sync.dma_start(b1s[:], b1s_d[:])
            b1n = pc.tile([16, 1], f32); nc.sync.dma_start(b1n[:], b1n_d[:])
            b2s = pc.tile([128, 1], f32); nc.sync.dma_start(b2s[:], b2s_d[:])
            b2n = pc.tile([16, 1], f32); nc.sync.dma_start(b2n[:], b2n_d[:])
            bdec = pc.tile([12, 1], f32); nc.sync.dma_start(bdec[:], bdec_d[:])
            b3 = pc.tile([3, 1], f32); nc.sync.dma_start(b3[:], b3_d[:])
            ident = pc.tile([128, 128], f32)
            masks.make_identity(nc, ident[:])
            identr = pc.tile([128, 128], f32r)
            nc.vector.tensor_copy(identr[:], ident[:])

            # ------- persistent across phases -------
            w_f32 = pp.tile([128, 32 * 128], f32)       # token-major softmax weights
            vnew = pp.tile([32, 128], f32r)

            ps_ag = ps_ag_pool.tile([32, 160], f32)      # [A | G] accumulator

            # =====================  ENCODER  =====================
            with (
                tc.tile_pool(name="pE", bufs=1) as pE,
                tc.tile_pool(name="pEd", bufs=1) as pEd,
                tc.tile_pool(name="pst", bufs=2) as pst,
                tc.tile_pool(name="psA", bufs=1, space="PSUM") as psA,
                tc.tile_pool(name="psB", bufs=1, space="PSUM") as psB,
                tc.tile_pool(name="psC2", bufs=2, space="PSUM") as psC2,
            ):
                tstT = pE.tile([128, 128 * 128], bf16)   # t*^T, chunk c at cols 128c
                w_bf = pE.tile([128, 32 * 128], bf16)

                for s in range(NSTRIP):
                    y0 = 32 * s
                    # ---- im2col for conv1 (rows y0-1 .. y0+32 of conv1 output) ----
                    im1 = pEd.tile([27, 34 * 256], f32r, tag="im1")
                    for ky in range(3):
                        for kx in range(3):
                            t = ky * 3 + kx
                            r_lo = max(0, 2 - y0 - ky)
                            r_hi = min(34, 258 - y0 - ky)
                            c_lo = max(0, 1 - kx)
                            c_hi = min(256, 257 - kx)
                            nr, ncol = r_hi - r_lo, c_hi - c_lo
                            src = x_d[0:3,
                                      y0 - 2 + r_lo + ky: y0 - 2 + r_hi + ky,
                                      c_lo + kx - 1: c_hi + kx - 1]
                            nc.sync.dma_start(
                                im1[3 * t:3 * t + 3, :]
                                .rearrange("p (r c) -> p r c", r=34)[:, r_lo:r_hi, c_lo:c_hi],
                                src)
                            # zero fills
                            if r_lo > 0:
                                nc.sync.dma_start(
                                    im1[3 * t:3 * t + 3, 0:r_lo * 256],
                                    zer_d[0:3, 0:r_lo * 256] if r_lo * 256 <= 1024 else zer_d[0:3, 0:1024])
                            if r_hi < 34:
                                nrm = (34 - r_hi) * 256
                                nc.sync.dma_start(
                                    im1[3 * t:3 * t + 3, r_hi * 256: 34 * 256],
                                    zer_d[0:3, 0:nrm])
                            if c_lo > 0:
                                nc.sync.dma_start(
                                    im1[3 * t:3 * t + 3, :]
                                    .rearrange("p (r c) -> p r c", r=34)[:, :, 0:1],
                                    zer_d[0:3, 0:34].rearrange("p (r c) -> p r c", c=1))
                            if c_hi < 256:
                                nc.sync.dma_start(
                                    im1[3 * t:3 * t + 3, :]
                                    .rearrange("p (r c) -> p r c", r=34)[:, :, 255:256],
                                    zer_d[0:3, 0:34].rearrange("p (r c) -> p r c", c=1))

                    # ---- conv1 wide + narrow (17 chunks of 512 px = 2 rows) ----
                    a1s = pEd.tile([128, 34 * RW], f32r, tag="a1s")
                    # a1n flat, col-deinterleaved: row lr at [lr*260, (lr+1)*260):
                    # evens (cx=2e) at +e, odds (cx=2j+1) at +130+j
                    a1n = pE.tile([16, 34 * 260], f32r, tag="a1n")
                    for i in range(17):
                        c1 = psB.tile([128, 512], f32, tag="c1s")
                        nc.tensor.matmul(c1[:], w1s[:], im1[:, 512 * i:512 * (i + 1)],
                                         start=True, stop=True)
                        nc.scalar.activation(
                            a1s[:, :].rearrange("p (r c) -> p r c", c=RW)
                            [:, 2 * i:2 * i + 2, 1:257],
                            c1[:].rearrange("p (r c) -> p r c", r=2),
                            AF.Silu, bias=b1s[:])
                        c1n = psB.tile([16, 512], f32, tag="c1n")
                        nc.tensor.matmul(c1n[:], w1n[:], im1[:, 512 * i:512 * (i + 1)],
                                         start=True, stop=True)
                        # ACT writes de-interleaved: psum col (r,e,pe) ->
                        # a1n offset r*260 + pe*130 + e
                        nc.scalar.activation(
                            _mk_ap(a1n[:], 520 * i,
                                   [[260, 2], [1, 128], [130, 2]]),
                            c1n[:].rearrange("p (r e two) -> p r e two", r=2, two=2),
                            AF.Silu, bias=b1n[:])
                    # zero pads of a1s / a1n
                    nc.sync.dma_start(
                        a1s[:].rearrange("p (r c) -> p r c", c=RW)[:, :, 0:1],
                        zer_d[:, 0:34].rearrange("p (r c) -> p r c", c=1))
                    nc.sync.dma_start(
                        a1s[:].rearrange("p (r c) -> p r c", c=RW)[:, :, 257:258],
                        zer_d[:, 0:34].rearrange("p (r c) -> p r c", c=1))
                    if s == 0:      # conv2 zero-pad at image top: a1 row lr=0
                        nc.sync.dma_start(a1s[:, 1:257], zer_d[:, 0:256])
                        nc.sync.dma_start(a1n[:, 0:260], zer_d[0:16, 0:260])
                    if s == NSTRIP - 1:  # bottom: lr=33
                        nc.sync.dma_start(
                            a1s[:, 33 * RW + 1:33 * RW + 257], zer_d[:, 0:256])
                        nc.sync.dma_start(
                            a1n[:, 33 * 260:34 * 260], zer_d[0:16, 0:260])

                    # ---- im2col for conv2 narrow (stride 2) ----
                    im2A = pE.tile([128, 2048], f32r, tag="im2A")
                    im2B = pE.tile([16, 2048], f32r, tag="im2B")
                    for ky in range(3):
                        for kx in range(3):
                            t = ky * 3 + kx
                            dstt = im2A if t < 8 else im2B
                            prow = 16 * t if t < 8 else 0
                            dst = dstt[prow:prow + 16, :] \
                                .rearrange("p (r c) -> p r c", c=128)
                            if kx == 1:    # evens e=ox
                                src = _mk_ap(a1n[:], ky * 260,
                                             [[520, 16], [1, 128]])
                                nc.sync.dma_start(dst[:, :, :].opt(), src)
                            elif kx == 2:  # odds j=ox
                                src = _mk_ap(a1n[:], ky * 260 + 130,
                                             [[520, 16], [1, 128]])
                                nc.sync.dma_start(dst[:, :, :].opt(), src)
                            else:          # kx=0: odds j=ox-1; col ox=0 zero
                                src = _mk_ap(a1n[:], ky * 260 + 130,
                                             [[520, 16], [1, 127]])
                                nc.sync.dma_start(dst[:, :, 1:128].opt(), src)
                                nc.sync.dma_start(
                                    dst[:, :, 0:1],
                                    zer_d[0:16, 0:16].rearrange(
                                        "p (r c) -> p r c", c=1))

                    # ---- conv2 narrow -> z ----
                    z_fl = pE.tile([16, 2048], f32r, tag="z")
                    for q in range(4):
                        c2n = psB.tile([16, 512], f32, tag="c2n")
                        nc.tensor.matmul(c2n[:], w2nA[:], im2A[:, 512 * q:512 * (q + 1)],
                                         start=True, stop=False)
                        nc.tensor.matmul(c2n[:], w2nB[:], im2B[:, 512 * q:512 * (q + 1)],
                                         start=False, stop=True)
                        nc.scalar.activation(z_fl[:, 512 * q:512 * (q + 1)], c2n[:],
                                             AF.Silu, bias=b2n[:])

                    # ---- conv2 wide + silu + transpose ----
                    ps_log = psA.tile([128, 512], f32, tag="pslog")
                    for q in range(4):
                        c2 = psC2.tile([128, 512], f32, tag="c2s")
                        for t9 in range(9):
                            ky, kx = t9 // 3, t9 % 3
                            rhs = a1s[:, :].rearrange("p (r c) -> p r c", c=RW)[
                                :, 8 * q + ky: 8 * q + ky + 8: 2, kx: kx + 256: 2]
                            nc.tensor.matmul(c2[:], w2s[:, 128 * t9:128 * (t9 + 1)],
                                             rhs, start=(t9 == 0), stop=(t9 == 8))
                        ts_t = pst.tile([128, 512], f32r, tag="tst")
                        nc.scalar.activation(ts_t[:], c2[:], AF.Silu, bias=b2s[:])
                        ps_tr = psB.tile([128, 512], f32r, tag="pstr")
                        for j in range(4):
                            nc.tensor.transpose(ps_tr[:, 128 * j:128 * (j + 1)],
                                                ts_t[:, 128 * j:128 * (j + 1)], identr[:])
                        nc.vector.tensor_copy(
                            tstT[:, 2048 * s + 512 * q: 2048 * s + 512 * (q + 1)],
                            ps_tr[:].bitcast(f32))

                        # logits for the 4 chunks of this q
                        for j in range(4):
                            c = 16 * s + 4 * q + j
                            nc.tensor.matmul(
                                ps_log[:, 32 * (4 * q + j):32 * (4 * q + j) + 32],
                                z_fl[0:16, 512 * q + 128 * j: 512 * q + 128 * (j + 1)],
                                ckt[:], start=True, stop=True)

                    # ---- softmax over 32 slots (free dim), 16 chunks at once ----
                    e_st = pst.tile([128, 512], f32, tag="est")
                    nc.scalar.activation(e_st[:], ps_log[:], AF.Exp)
                    den = pst.tile([128, 16], f32, tag="den")
                    nc.vector.tensor_reduce(
                        den[:], e_st[:].rearrange("p (c k) -> p c k", k=32),
                        mybir.AxisListType.X, ALU.add)
                    rec = pst.tile([128, 16], f32, tag="rec")
                    nc.vector.reciprocal(rec[:], den[:])
                    wslice = w_f32[:, 512 * s:512 * (s + 1)]
                    nc.vector.tensor_tensor(
                        wslice.rearrange("p (c k) -> p c k", k=32),
                        e_st[:].rearrange("p (c k) -> p c k", k=32),
                        rec[:].rearrange("p (c k) -> p c k", k=1).broadcast_to([128, 16, 32]),
                        ALU.mult)
                    nc.vector.tensor_copy(w_bf[:, 512 * s:512 * (s + 1)], wslice)

                    # ---- A|G accumulation over this strip's 16 chunks ----
                    for j in range(16):
                        c = 16 * s + j
                        lhs = w_bf[:, 32 * c:32 * c + 32]
                        nc.tensor.matmul(ps_ag[:, 0:128], lhs,
                                         tstT[:, 128 * c:128 * (c + 1)],
                                         start=(c == 0), stop=(c == 127))
                        nc.tensor.matmul(ps_ag[:, 128:160], lhs, lhs,
                                         start=(c == 0), stop=(c == 127))

                # ---- dV + collective (still inside encoder pools) ----
                a_sb = pst.tile([32, 128], f32, tag="asb", bufs=1)
                nc.vector.tensor_copy(a_sb[:], ps_ag[:, 0:128])
                g_sb = pst.tile([32, 32], f32r, tag="gsb", bufs=1)
                nc.vector.tensor_copy(g_sb[:], ps_ag[:, 128:160])
                ps_gv_t = psC2.tile([128, 512], f32, tag="c2s", name="psgv")
                ps_gv = ps_gv_t[0:32, 0:128]
                nc.tensor.matmul(ps_gv[:], g_sb[:], vmat[:], start=True, stop=True)
                dv_sb = pst.tile([32, 128], f32, tag="dvsb", bufs=1)
                nc.vector.tensor_sub(dv_sb[:], a_sb[:], ps_gv[:])
                dv_in = pdram.tile([32, 128], f32)
                dv_out = pdram.tile([32 * N_CORES, 128], f32)
                nc.sync.dma_start(dv_in[:], dv_sb[:])
                nc.gpsimd.collective_compute(
                    "AllGather", ALU.bypass,
                    replica_groups=[list(range(N_CORES))],
                    ins=[dv_in.opt()], outs=[dv_out.opt()])
                gath = pst.tile([32, 8 * 128], f32, tag="gath", bufs=1)
                nc.sync.dma_start(
                    gath[:].rearrange("p (r c) -> p r c", r=N_CORES),
                    dv_out[:].rearrange("(r p) c -> p r c", p=32))
                nc.vector.tensor_add(gath[:, 0:512], gath[:, 0:512], gath[:, 512:1024])
                nc.vector.tensor_add(gath[:, 0:256], gath[:, 0:256], gath[:, 256:512])
                nc.vector.tensor_add(gath[:, 0:128], gath[:, 0:128], gath[:, 128:256])
                nc.vector.scalar_tensor_tensor(
                    vnew[:], gath[:, 0:128], ALPHA, vmat[:],
                    op0=ALU.mult, op1=ALU.add)

            # =====================  DECODER  =====================
            with (
                tc.tile_pool(name="pD", bufs=1) as pD,
                tc.tile_pool(name="pDd", bufs=1) as pDd,
                tc.tile_pool(name="pst2", bufs=3) as pst2,
                tc.tile_pool(name="psC", bufs=2, space="PSUM") as psC,
                tc.tile_pool(name="psD", bufs=1, space="PSUM") as psD,
            ):
                # ---- w slot-major via PE transpose ----
                w_sT = pD.tile([32, 16384], f32r)
                for g in range(32):           # 4 chunks per psum bank
                    ps_wt = psD.tile([32, 512], f32, tag="pswt")
                    for j in range(4):
                        c = 4 * g + j
                        nc.tensor.transpose(ps_wt[:, 128 * j:128 * (j + 1)],
                                            w_f32[:, 32 * c:32 * c + 32], ident[:])
                    nc.vector.tensor_copy(w_sT[:, 512 * g:512 * (g + 1)], ps_wt[:])

                # ---- t_read^T -> d0m (padded [130,130]) ----
                d0m = pD.tile([128, 130 * 130], f32r)
                nc.sync.dma_start(d0m[:, 0:130], zer_d[:, 0:130])
                nc.sync.dma_start(d0m[:, 129 * 130:130 * 130], zer_d[:, 0:130])
                nc.sync.dma_start(
                    d0m[:].rearrange("p (r c) -> p r c", c=130)[:, :, 0:1],
                    zer_d[:, 0:130].rearrange("p (r c) -> p r c", c=1))
                nc.sync.dma_start(
                    d0m[:].rearrange("p (r c) -> p r c", c=130)[:, :, 129:130],
                    zer_d[:, 0:130].rearrange("p (r c) -> p r c", c=1))
                for q in range(32):
                    ps_rd = psC.tile([128, 512], f32, tag="psrd")
                    nc.tensor.matmul(ps_rd[:], vnew[:], w_sT[:, 512 * q:512 * (q + 1)],
                                     start=True, stop=True)
                    nc.vector.tensor_copy(
                        d0m[:].rearrange("p (r c) -> p r c", c=130)
                        [:, 4 * q + 1:4 * q + 5, 1:129],
                        ps_rd[:].rearrange("p (r c) -> p r c", r=4))

                # ---- deconv -> silu -> img12 in DRAM (parity planes, padded) ----
                # img12_d[(a*2+b)*3+o, I+1, J+1] = silu(deconv)[o, 2I+a, 2J+b]
                img12_d = pdram.tile([12, 130, 130], f32r, name="img12d")
                nc.sync.dma_start(img12_d[:, 0, :], zer_d[0:12, 0:130])
                nc.sync.dma_start(img12_d[:, 129, :], zer_d[0:12, 0:130])
                nc.sync.dma_start(
                    img12_d[:, :, 0:1],
                    zer_d[0:12, 0:130].rearrange("p (r c) -> p r c", c=1))
                nc.sync.dma_start(
                    img12_d[:, :, 129:130],
                    zer_d[0:12, 0:130].rearrange("p (r c) -> p r c", c=1))
                for q in range(32):
                    ps_dec = psC.tile([12, 512], f32, tag="psdec")
                    for t9 in range(9):
                        dy, dx = t9 // 3 - 1, t9 % 3 - 1
                        rhs = d0m[:].rearrange("p (r c) -> p r c", c=130)[
                            :, 4 * q + 1 + dy:4 * q + 5 + dy, 1 + dx:129 + dx]
                        nc.tensor.matmul(ps_dec[:], decw[:, 12 * t9:12 * (t9 + 1)],
                                         rhs, start=(t9 == 0), stop=(t9 == 8))
                    stg = pst2.tile([12, 512], f32r, tag="stdec")
                    nc.scalar.activation(stg[:], ps_dec[:], AF.Silu, bias=bdec[:])
                    nc.sync.dma_start(
                        img12_d[:, 1 + 4 * q:5 + 4 * q, 1:129],
                        stg[:].rearrange("p (r c) -> p r c", r=4))

                # ---- conv3 per output parity class ----
                # out(2I'+a', 2J'+b') reads img(2I'+a'+ky-1, 2J'+b'+kx-1):
                #   parity (a,b), halfres (I'+dy, J'+dx)
                for ap_ in range(2):
                    for q4 in range(4):          # quarter planes: I' in [32q4, 32q4+32)
                        I0 = 32 * q4
                        imcls = []
                        for b_ in range(2):
                            imc = pDd.tile([27, 32 * 128], f32r,
                                           tag=f"im3c{b_}", name=f"im3c{b_}")
                            for ky in range(3):
                                va = ap_ + ky - 1
                                a = va % 2
                                dy = (va - a) // 2
                                for kx in range(3):
                                    vb = b_ + kx - 1
                                    b = vb % 2
                                    dx = (vb - b) // 2
                                    t = ky * 3 + kx
                                    prow = (a * 2 + b) * 3
                                    nc.sync.dma_start(
                                        imc[3 * t:3 * t + 3, :]
                                        .rearrange("p (r c) -> p r c", c=128),
                                        img12_d[prow:prow + 3,
                                                I0 + dy + 1:I0 + dy + 33,
                                                dx + 1:dx + 129])
                            imcls.append(imc)
                        for ch in range(8):      # 4 I'-rows per chunk
                            rb = pst2.tile([3, 1024], f32, tag="rb")
                            for b_ in range(2):
                                c3 = psC.tile([3, 512], f32, tag="c3")
                                nc.tensor.matmul(
                                    c3[:], cw3[:],
                                    imcls[b_][:, 512 * ch:512 * (ch + 1)],
                                    start=True, stop=True)
                                nc.scalar.activation(
                                    _mk_ap(rb[:], b_, [[256, 4], [2, 128]]),
                                    c3[:].rearrange("p (r c) -> p r c", r=4),
                                    AF.Silu, bias=b3[:])
                            y0o = 2 * (I0 + 4 * ch) + ap_
                            nc.sync.dma_start(
                                out_d[0:3, y0o:y0o + 7:2, :],
                                rb[:].rearrange("p (r c) -> p r c", c=256))

    nc.compile()
    return nc


def _group_runs(u_lo, u_hi, ibase, ioff):
    """Split u-range so (ibase + u + ioff)//16 is constant per run."""
    runs = []
    u = u_lo
    while u < u_hi:
        grp = (ibase + u + ioff) // 16
        ue = u
        while ue < u_hi and (ibase + ue + ioff) // 16 == grp:
            ue += 1
        runs.append((u, ue))
        u = ue
    return runs


def _prep_weights(i):
    """Host-side weight layout prep. i = dict of full inputs."""
    f = np.float32
    w1s = np.ascontiguousarray(
        i['e0s_w1'].transpose(2, 3, 1, 0).reshape(27, 128)).astype(f)
    w1n = np.ascontiguousarray(
        i['e0n_w1'].transpose(2, 3, 1, 0).reshape(27, 16)).astype(f)
    w2s = np.ascontiguousarray(
        i['e0s_w2'].transpose(1, 2, 3, 0).reshape(128, 9 * 128)).astype(f)
    w2n = np.ascontiguousarray(
        i['e0n_w2'].transpose(2, 3, 1, 0).reshape(9, 16, 16)).astype(f)
    w2nA = w2n[0:8].reshape(128, 16).copy()
    w2nB = w2n[8].copy()
    ckt = (i['cell_k'].T * np.float32(0.25)).astype(f).copy()   # [16,32], /sqrt(16)
    vmat = i['cell_v'].astype(f).copy()
    # deconv: shift s=(dy,dx); decw[s][c, (a*2+b)*3+o] = W[c,o,ky(a,u),kx(b,v)]
    dw = i['d0_dw']  # [128, 3, 4, 4]
    decw = np.zeros((9, 128, 12), f)  # reshaped to [128, 108] below
    for a in range(2):
        for u in range(2):
            ky = (1, 3)[u] if a == 0 else (0, 2)[u]
            dy = (0, -1)[u] if a == 0 else (1, 0)[u]
            for b in range(2):
                for v in range(2):
                    kx = (1, 3)[v] if b == 0 else (0, 2)[v]
                    dx = (0, -1)[v] if b == 0 else (1, 0)[v]
                    sidx = (dy + 1) * 3 + (dx + 1)
                    for o in range(3):
                        decw[sidx, :, (a * 2 + b) * 3 + o] += dw[:, o, ky, kx]
    cw3 = np.ascontiguousarray(
        i['d0_cw'].transpose(2, 3, 1, 0).reshape(27, 3)).astype(f)
    bdec = np.zeros((12, 1), f)
    for ab in range(4):
        bdec[3 * ab:3 * ab + 3, 0] = i['d0_db']
    return dict(
        w1s=w1s, w1n=w1n, w2s=w2s, w2nA=w2nA, w2nB=w2nB, ckt=ckt, vmat=vmat,
        decw=np.ascontiguousarray(decw.transpose(1, 0, 2).reshape(128, 108)),
        cw3=cw3,
        b1s=i['e0s_b1'].reshape(128, 1).astype(f),
        b1n=i['e0n_b1'].reshape(16, 1).astype(f),
        b2s=i['e0s_b2'].reshape(128, 1).astype(f),
        b2n=i['e0n_b2'].reshape(16, 1).astype(f),
        bdec=bdec, b3=i['d0_cb'].reshape(3, 1).astype(f),
        zer=np.zeros((128, 1024), f),
    )


_last = {}


def last_exec_ns():
    return _last.get('ns')


def _get_runner():
    """Cached jitted SPMD callable over 8 cores (traced once)."""
    if 'runner' in _cache:
        return _cache['runner']
    import jax
    from jax.sharding import Mesh, PartitionSpec
    from jax.experimental.shard_map import shard_map
    from concourse import bass2jax, mybir as _mb
    nc = _cache['nc']
    bass2jax.install_neuronx_cc_hook()
    partition_name = nc.partition_id_tensor.name if nc.partition_id_tensor else None
    in_names, out_names, out_avals, zero_outs = [], [], [], []
    for alloc in nc.m.functions[0].allocations:
        if not isinstance(alloc, _mb.MemoryLocationSet):
            continue
        name = alloc.memorylocations[0].name
        if alloc.kind == "ExternalInput":
            if name != partition_name:
                in_names.append(name)
        elif alloc.kind == "ExternalOutput":
            shape = tuple(alloc.tensor_shape)
            dtype = _mb.dt.np(alloc.dtype)
            out_names.append(name)
            out_avals.append(jax.core.ShapedArray(shape, dtype))
            zero_outs.append(np.zeros(shape, dtype))
    n_params = len(in_names)
    n_outs = len(out_avals)
    all_names = list(in_names) + list(out_names)
    if partition_name is not None:
        all_names.append(partition_name)
    donate = tuple(range(n_params, n_params + n_outs))

    def _body(*args):
        operands = list(args)
        if partition_name is not None:
            operands.append(bass2jax.partition_id_tensor())
        outs = bass2jax._bass_exec_p.bind(
            *operands, out_avals=tuple(out_avals), in_names=tuple(all_names),
            out_names=tuple(out_names), lowering_input_output_aliases=(),
            sim_require_finite=True, sim_require_nnan=True, nc=nc)
        return tuple(outs)

    devices = jax.devices()[:N_CORES]
    mesh = Mesh(np.asarray(devices), ("core",))
    sharded = jax.jit(
        shard_map(_body, mesh=mesh,
                  in_specs=(PartitionSpec("core"),) * (n_params + n_outs),
                  out_specs=(PartitionSpec("core"),) * n_outs,
                  check_rep=False),
        keep_unused=True)

    from jax.sharding import NamedSharding
    sh = NamedSharding(mesh, PartitionSpec("core"))
    _cache['sharding'] = sh
    _cache['devices'] = devices
    _cache['runner'] = (sharded, in_names, out_names, out_avals, zero_outs)
    return _cache['runner']


def _make_global(per_core_arrs):
    """Assemble a sharded global array from per-core numpy shards (no
    on-device slicing)."""
    import jax
    sh = _cache['sharding']
    devices = _cache['devices']
    a0 = np.asarray(per_core_arrs[0])
    global_shape = (len(per_core_arrs) * a0.shape[0], *a0.shape[1:])
    bufs = [jax.device_put(np.ascontiguousarray(a), d)
            for a, d in zip(per_core_arrs, devices)]
    return jax.make_array_from_single_device_arrays(global_shape, sh, bufs)


def _run_fast(in_maps):
    import jax
    sharded, in_names, out_names, out_avals, zero_outs = _get_runner()
    if 'dev_zeros' not in _cache:
        _cache['dev_zeros'] = [
            _make_global([np.zeros(z.shape, z.dtype)] * N_CORES)
            for z in zero_outs]
    n_cores = len(in_maps)
    gin = [_make_global([in_maps[c][nm] for c in range(n_cores)])
           for nm in in_names]
    outs = sharded(*gin, *_cache['dev_zeros'])
    return [{nm: np.asarray(outs[i]).reshape(n_cores, *out_avals[i].shape)[c]
             for i, nm in enumerate(out_names)} for c in range(n_cores)]


def _get_chain_runner(n_chain):
    """Jitted callable running the kernel n_chain times serially on-device
    (each iteration's 'out' feeds the next one's out-buffer operand)."""
    key = f'chain{n_chain}'
    if key in _cache:
        return _cache[key]
    import jax
    from jax.sharding import Mesh, PartitionSpec
    from jax.experimental.shard_map import shard_map
    from concourse import bass2jax, mybir as _mb
    nc = _cache['nc']
    bass2jax.install_neuronx_cc_hook()
    partition_name = nc.partition_id_tensor.name if nc.partition_id_tensor else None
    in_names, out_names, out_avals = [], [], []
    for alloc in nc.m.functions[0].allocations:
        if not isinstance(alloc, _mb.MemoryLocationSet):
            continue
        name = alloc.memorylocations[0].name
        if alloc.kind == "ExternalInput":
            if name != partition_name:
                in_names.append(name)
        elif alloc.kind == "ExternalOutput":
            out_names.append(name)
            out_avals.append(jax.core.ShapedArray(
                tuple(alloc.tensor_shape), _mb.dt.np(alloc.dtype)))
    n_params = len(in_names)
    all_names = list(in_names) + list(out_names)
    if partition_name is not None:
        all_names.append(partition_name)
    oi = out_names.index("out")

    def _body(*args):
        ins = list(args[:n_params])
        outbufs = list(args[n_params:])
        for _ in range(n_chain):
            operands = ins + outbufs
            if partition_name is not None:
                operands.append(bass2jax.partition_id_tensor())
            res = bass2jax._bass_exec_p.bind(
                *operands, out_avals=tuple(out_avals), in_names=tuple(all_names),
                out_names=tuple(out_names), lowering_input_output_aliases=(),
                sim_require_finite=True, sim_require_nnan=True, nc=nc)
            outbufs[oi] = res[oi]       # serialize iterations
        return tuple(res)

    devices = jax.devices()[:N_CORES]
    mesh = Mesh(np.asarray(devices), ("core",))
    n_outs = len(out_avals)
    fn = jax.jit(
        shard_map(_body, mesh=mesh,
                  in_specs=(PartitionSpec("core"),) * (n_params + n_outs),
                  out_specs=(PartitionSpec("core"),) * n_outs,
                  check_rep=False),
        keep_unused=True)
    _cache[key] = fn
    return fn


def _build_tiny():
    nc = bacc.Bacc("TRN2", target_bir_lowering=False, name="tiny")
    xi = nc.dram_tensor("xi", [128, 128], f32, kind="ExternalInput")
    xo = nc.dram_tensor("xo", [128, 128], f32, kind="ExternalOutput")
    with tile.TileContext(nc) as tc:
        with tc.tile_pool(name="sb", bufs=1) as sb:
            t = sb.tile([128, 128], f32)
            nc.sync.dma_start(t[:], xi[:])
            nc.sync.dma_start(xo[:], t[:])
    nc.compile()
    return nc


def bench_hw(n_iter=12, **inputs):
    """Estimate device exec time: full-kernel min wall minus trivial-kernel
    min wall (same 8-core dispatch path)."""
    import time as _t, jax
    from jax.sharding import Mesh, PartitionSpec
    from jax.experimental.shard_map import shard_map
    from concourse import bass2jax
    if 'nc' not in _cache:
        _cache['nc'] = _build()
    shared = _prep_weights({k: np.asarray(v) for k, v in inputs.items()})
    x = np.asarray(inputs['x'], dtype=np.float32)
    in_maps = [dict(shared, x=np.ascontiguousarray(x[c])) for c in range(N_CORES)]
    sharded, in_names, out_names, out_avals, zero_outs = _get_runner()
    gin = [_make_global([in_maps[c][nm] for c in range(N_CORES)])
           for nm in in_names]
    gz = [_make_global([np.zeros(z.shape, z.dtype)] * N_CORES)
          for z in zero_outs]

    def mintime(fn, args):
        ts = []
        for _ in range(n_iter):
            t0 = _t.perf_counter()
            o = fn(*args)
            jax.block_until_ready(o)
            ts.append(_t.perf_counter() - t0)
        return min(ts), ts

    tfull, ts_full = mintime(sharded, (*gin, *gz))

    if 'tiny_fn' not in _cache:
        ncT = _build_tiny()
        bass2jax.install_neuronx_cc_hook()
        pn = ncT.partition_id_tensor.name if ncT.partition_id_tensor else None

        def _tb(xi, xoz):
            ops = [xi, xoz]
            if pn is not None:
                ops.append(bass2jax.partition_id_tensor())
            names = ["xi", "xo"] + ([pn] if pn else [])
            return tuple(bass2jax._bass_exec_p.bind(
                *ops,
                out_avals=(jax.core.ShapedArray((128, 128), np.float32),),
                in_names=tuple(names), out_names=("xo",),
                lowering_input_output_aliases=(),
                sim_require_finite=True, sim_require_nnan=True, nc=ncT))
        mesh = Mesh(np.asarray(_cache['devices']), ("core",))
        _cache['tiny_fn'] = jax.jit(shard_map(
            _tb, mesh=mesh, in_specs=(PartitionSpec("core"),) * 2,
            out_specs=(PartitionSpec("core"),), check_rep=False),
            keep_unused=True)
        _cache['tiny_in'] = (
            _make_global([np.zeros((128, 128), np.float32)] * N_CORES),
            _make_global([np.zeros((128, 128), np.float32)] * N_CORES))
    ttiny, ts_tiny = mintime(_cache['tiny_fn'], _cache['tiny_in'])
    return max(0.0, tfull - ttiny), tfull, ttiny


def bench(n_iter=20, **inputs):
    """Min wall time of the on-device executable (inputs pre-staged)."""
    import time as _t, jax
    if 'nc' not in _cache:
        _cache['nc'] = _build()
    shared = _prep_weights({k: np.asarray(v) for k, v in inputs.items()})
    x = np.asarray(inputs['x'], dtype=np.float32)
    in_maps = [dict(shared, x=np.ascontiguousarray(x[c])) for c in range(N_CORES)]
    sharded, in_names, out_names, out_avals, zero_outs = _get_runner()
    gin = [_make_global([in_maps[c][nm] for c in range(N_CORES)])
           for nm in in_names]
    times = []
    for it in range(n_iter):
        t0 = _t.perf_counter()
        outs = sharded(*gin, *_cache['dev_zeros'])
        jax.block_until_ready(outs)
        times.append(_t.perf_counter() - t0)
    return min(times), times


def kernel(**inputs):
    if 'nc' not in _cache:
        _cache['nc'] = _build()
    nc = _cache['nc']
    shared = _prep_weights({k: np.asarray(v) for k, v in inputs.items()})
    x = np.asarray(inputs['x'], dtype=np.float32)
    in_maps = [dict(shared, x=np.ascontiguousarray(x[c])) for c in range(N_CORES)]
    res = _run_fast(in_maps)
    out = np.stack([res[c]["out"] for c in range(N_CORES)], axis=0)
    return out
